# revision 54
# baseline (speedup 1.0000x reference)
"""Trainium2 Bass kernel for MultiHeadAttention with physics bias.

Math (per reference):
    q = (Q @ Wq + bq) * scale          # scale folded into Wq/bq host-side
    k = (K @ Wk + bk)
    scores = q @ k^T + lambda * physics_bias
    attn = softmax(scores, axis=-1)    # exp without max-subtraction (scores are O(+-6))
    ctx = attn @ v ; out = Q + ctx @ Wo + bo

Sharding: the 24 (batch, head) pairs are split 3-per-core across 8 cores
(cores 0-3 -> batch 0, cores 4-7 -> batch 1; each takes 3 consecutive heads).
The device computes the O(S^2) part (scores, exp, rowsum, normalize) which is
memory-bound on the [B,H,S,S] bias read + attn write (~100 MB per core).
The O(S) tail (ctx = attn @ v, output projection, residual) is a ~1 GFLOP
epilogue computed on the host from the returned attn.

Device per (head, 128-query chunk):
    PE   : 4x bf16 matmul  q_chunk^T.T @ k^T -> PSUM [128,2048]   (K=5)
    DVE  : scores = (bias * lambda) + psum   (one scalar_tensor_tensor op)
    ACT  : E = exp(scores) in-place, accum_out = row sums
    DVE  : reciprocal; attn = E * (1/sum) in-place
    DMA  : bias in / attn out in 2 MB transfers (two chunks per transfer)
"""

import sys

import numpy as np

try:
    import concourse  # noqa: F401  (already importable via the site setup)
except ImportError:
    _TRN_REPO = "/opt/trn_rl_repo"
    if _TRN_REPO not in sys.path:
        sys.path.insert(0, _TRN_REPO)

B, S, D_MODEL, N_HEADS, D_K = 2, 2048, 64, 12, 5
D_INNER = N_HEADS * D_K          # 60
HPC = 3                          # heads per core
NCORES = 8
DA = D_MODEL + 1                 # augmented with a ones-row to fold biases bq/bk
BANK = 512                       # fp32 PSUM bank / max moving free dim
CHUNK = 128                      # query rows per softmax chunk (= partitions)
GROUP = 2                        # chunks per DMA transfer (2 MB)
GROUP_ROWS = GROUP * CHUNK
NGROUP = S // GROUP_ROWS         # transfers per head

_PROG = None                     # compiled program cache (module-level)


def _build_program():
    import concourse.bacc as bacc
    import concourse.bass as bass
    import concourse.mybir as mybir
    import concourse.tile as tile

    f32 = mybir.dt.float32
    f16 = mybir.dt.float16
    bf16 = mybir.dt.bfloat16
    AF = mybir.ActivationFunctionType
    OP = mybir.AluOpType

    nc = bacc.Bacc("TRN2", target_bir_lowering=False, debug=False,
                   num_devices=NCORES)

    xq = nc.dram_tensor("xq", [DA, S], bf16, kind="ExternalInput").ap()
    xk = nc.dram_tensor("xk", [DA, S], bf16, kind="ExternalInput").ap()
    wq = nc.dram_tensor("wq", [DA, HPC * D_K], bf16, kind="ExternalInput").ap()
    wk = nc.dram_tensor("wk", [DA, HPC * D_K], bf16, kind="ExternalInput").ap()
    # lamdiag[:, 0:128] = lambda * I[128]: bias-add via TensorE diagonal
    # matmul into the scores PSUM; lamdiag[:, 128] = a lambda column for the
    # DVE half of the bias-add (lambda=0.5/1.0 are exact in fp16)
    lamdiag = nc.dram_tensor("lamdiag", [CHUNK, CHUNK + 1], f16,
                             kind="ExternalInput").ap()
    # bias is loaded as fp16: halves the dominant read traffic; fp16's 2^-11
    # mantissa keeps the attn error ~1e-3 (bf16 would be ~1e-2)
    biasd = nc.dram_tensor("biasd", [HPC, S, S], f16, kind="ExternalInput").ap()
    # attn is stored as fp16 (host upcasts): halves the write traffic for
    # another ~5e-4 of rounding on values that are already ~1e-3 approximate
    attnd = nc.dram_tensor("attnd", [HPC, S, S], f16, kind="ExternalOutput").ap()

    with tile.TileContext(nc) as tc:
        with (
            tc.tile_pool(name="const", bufs=1) as const,
            tc.tile_pool(name="spsum", bufs=2, space=bass.MemorySpace.PSUM) as spool,
            tc.tile_pool(name="bias", bufs=8) as bpool,
            tc.tile_pool(name="attn", bufs=8) as apool,
            tc.tile_pool(name="stat", bufs=8) as stat,
        ):
            xq_sb = const.tile([DA, S], bf16, tag="xq", name="xq_sb")
            nc.sync.dma_start(xq_sb[:], xq[:])
            xk_sb = const.tile([DA, S], bf16, tag="xk", name="xk_sb")
            nc.sync.dma_start(xk_sb[:], xk[:])
            wq_sb = const.tile([DA, HPC * D_K], bf16, tag="wq", name="wq_sb")
            nc.sync.dma_start(wq_sb[:], wq[:])
            wk_sb = const.tile([DA, HPC * D_K], bf16, tag="wk", name="wk_sb")
            nc.sync.dma_start(wk_sb[:], wk[:])
            lam_sb = const.tile([CHUNK, CHUNK + 1], f16, tag="lam",
                                name="lam_sb")
            nc.sync.dma_start(lam_sb[:], lamdiag[:])

            # All q^T/k^T projections upfront (bf16, ~0.4us per matmul) so
            # head transitions never stall the scores/PSUM pipeline. Each is
            # then replicated at partition offsets 0/32/64/96 so the 4 score
            # matmuls of a chunk run in distinct PE row-groups concurrently
            # (their LDWEIGHTS overlap in-flight matmuls of other groups).
            qts, kts = [], []
            for h in range(HPC):
                qt = const.tile([CHUNK, S], bf16, tag=f"qt{h}", name=f"qt{h}")
                kt = const.tile([CHUNK, S], bf16, tag=f"kt{h}", name=f"kt{h}")
                for w_sb, x_sb, dst in ((wq_sb, xq_sb, qt), (wk_sb, xk_sb, kt)):
                    for j in range(S // BANK):
                        ps = spool.tile([CHUNK, S], f32, tag="ps", name="ps_proj")
                        nc.tensor.matmul(
                            ps[0:D_K, 0:BANK],
                            w_sb[:, h * D_K:(h + 1) * D_K],
                            x_sb[:, j * BANK:(j + 1) * BANK],
                            start=True, stop=True,
                        )
                        nc.vector.tensor_copy(
                            dst[0:D_K, j * BANK:(j + 1) * BANK],
                            ps[0:D_K, 0:BANK],
                        )
                    # replicate via GpSimd SWDGE: these wait on the projection
                    # compute, and on the sync ring they would block every
                    # bias load queued behind them (HWDGE strict FIFO)
                    for g in range(1, S // BANK):
                        nc.gpsimd.dma_start(
                            dst[32 * g:32 * g + D_K, :], dst[0:D_K, :])
                qts.append(qt)
                kts.append(kt)

            for h in range(HPC):
                qt, kt = qts[h], kts[h]
                for pr in range(NGROUP):
                    r0 = pr * GROUP_ROWS
                    bt = bpool.tile([CHUNK, GROUP * S], f16, tag="bt", name="bt")
                    nc.sync.dma_start(
                        bt[:],
                        biasd[h, r0:r0 + GROUP_ROWS, :].rearrange(
                            "(a p) n -> p a n", p=CHUNK),
                    )
                    for a in range(GROUP):
                        ps = spool.tile([CHUNK, S], f32, tag="ps", name="ps_sc")
                        # scores = q_chunk^T.T @ k^T
                        NB = S // BANK
                        PE_B = 2  # banks getting the bias via PE; rest on DVE
                        c0 = (pr * GROUP + a) * CHUNK
                        for j in range(NB):
                            nc.tensor.matmul(
                                ps[:, j * BANK:(j + 1) * BANK],
                                qt[32 * j:32 * j + D_K, c0:c0 + CHUNK],
                                kt[32 * j:32 * j + D_K,
                                   j * BANK:(j + 1) * BANK],
                                start=True, stop=(j >= PE_B),
                                tile_position=(32 * j, 0),
                            )
                        # += lambda * bias: banks 0..PE_B-1 via the diagonal
                        # matmul, the rest via DVE in-place into PSUM — splits
                        # the per-matmul LDWEIGHTS toll between two engines
                        for j in range(PE_B):
                            nc.tensor.matmul(
                                ps[:, j * BANK:(j + 1) * BANK],
                                lam_sb[:, 0:CHUNK],
                                bt[:, a * S + j * BANK:a * S + (j + 1) * BANK],
                                start=False, stop=True,
                            )
                        nc.vector.scalar_tensor_tensor(
                            ps[:, PE_B * BANK:],
                            bt[:, a * S + PE_B * BANK:(a + 1) * S],
                            lam_sb[:, CHUNK:CHUNK + 1],
                            ps[:, PE_B * BANK:],
                            op0=OP.mult, op1=OP.add,
                        )
                        # per-chunk attn tile: Tile tracks deps per tile, so a
                        # shared group tile would serialize chunk a=1 behind
                        # chunk a=0's normalize
                        at = apool.tile([CHUNK, S], f16, tag="at", name="at")
                        sm = stat.tile([CHUNK, 1], f32, tag="sm", name="sm")
                        nc.scalar.activation(at[:], ps[:], AF.Exp,
                                             accum_out=sm[:])
                        rc = stat.tile([CHUNK, 1], f32, tag="rc", name="rc")
                        nc.vector.reciprocal(rc[:], sm[:])
                        nc.vector.tensor_scalar_mul(at[:], at[:], rc[:])
                        # store via GpSimd SWDGE: its own DMA queue (loads
                        # ride the SP queue; fp16 both ways keeps the queues
                        # balanced) and an idle engine's stream (a store's
                        # sem-wait on ACT/SP would block exps / loads behind
                        # it)
                        nc.gpsimd.dma_start(
                            attnd[h, r0 + a * CHUNK:r0 + (a + 1) * CHUNK, :],
                            at[:],
                        )

    nc.compile()
    return nc


def _get_program():
    global _PROG
    if _PROG is None:
        _PROG = _build_program()
    return _PROG


def _make_in_maps(Q, K, physics_bias, lambda_scalar, Wq, bq, Wk, bk):
    import ml_dtypes
    bf16 = ml_dtypes.bfloat16
    scale = float(1.0 / np.sqrt(np.float32(D_K)).astype(np.float32))
    ones_row = np.ones((1, S), np.float32)
    lam_v = np.float32(np.asarray(lambda_scalar))
    lam_diag = np.concatenate(
        [np.eye(CHUNK, dtype=np.float32) * lam_v,
         np.full((CHUNK, 1), lam_v, np.float32)], axis=1).astype(np.float16)

    in_maps = []
    for c in range(NCORES):
        b = c // (N_HEADS // HPC)
        h0 = (c % (N_HEADS // HPC)) * HPC
        cols = slice(h0 * D_K, (h0 + HPC) * D_K)
        wq_aug = np.ascontiguousarray(
            np.vstack([Wq[:, cols], bq[cols][None, :]]) * np.float32(scale),
            np.float32)
        wk_aug = np.ascontiguousarray(
            np.vstack([Wk[:, cols], bk[cols][None, :]]), np.float32)
        in_maps.append({
            "xq": np.ascontiguousarray(
                np.vstack([Q[b].T, ones_row]).astype(bf16)),
            "xk": np.ascontiguousarray(
                np.vstack([K[b].T, ones_row]).astype(bf16)),
            "wq": wq_aug.astype(bf16),
            "wk": wk_aug.astype(bf16),
            "lamdiag": lam_diag,
            "biasd": np.ascontiguousarray(
                physics_bias[b, h0:h0 + HPC].astype(np.float16)),
        })
    return in_maps


def _run_device(in_maps, trace=False):
    from concourse.bass_utils import run_bass_kernel_spmd
    nc = _get_program()
    return run_bass_kernel_spmd(nc, in_maps, list(range(NCORES)), trace=trace)


def _host_tail(Q, V, Wv, bv, Wo, bo, attn):
    """ctx = attn @ v per head; out = Q + ctx @ Wo + bo  (all fp32 on host)."""
    v = V.astype(np.float32) @ Wv.astype(np.float32) + bv.astype(np.float32)
    # [B,S,60] -> [B,H,S,5]
    v_heads = np.ascontiguousarray(
        v.reshape(B, S, N_HEADS, D_K).transpose(0, 2, 1, 3))
    ctx = np.matmul(attn, v_heads)                       # [B,H,S,5]
    ctx = ctx.transpose(0, 2, 1, 3).reshape(B, S, D_INNER)
    out = Q.astype(np.float32) + (ctx @ Wo.astype(np.float32)
                                  + bo.astype(np.float32))
    return out.astype(np.float32)


def kernel(Q, K, V, physics_bias, lambda_scalar, Wq, bq, Wk, bk, Wv, bv, Wo, bo,
           _trace=False, _return_results=False):
    Q = np.asarray(Q, np.float32)
    K = np.asarray(K, np.float32)
    V = np.asarray(V, np.float32)
    physics_bias = np.asarray(physics_bias, np.float32)
    Wq = np.asarray(Wq, np.float32)
    bq = np.asarray(bq, np.float32)
    Wk = np.asarray(Wk, np.float32)
    bk = np.asarray(bk, np.float32)
    Wv = np.asarray(Wv, np.float32)
    bv = np.asarray(bv, np.float32)
    Wo = np.asarray(Wo, np.float32)
    bo = np.asarray(bo, np.float32)

    in_maps = _make_in_maps(Q, K, physics_bias, lambda_scalar, Wq, bq, Wk, bk)
    br = _run_device(in_maps, trace=_trace)

    attn = np.empty((B, N_HEADS, S, S), np.float32)
    for c in range(NCORES):
        b = c // (N_HEADS // HPC)
        h0 = (c % (N_HEADS // HPC)) * HPC
        attn[b, h0:h0 + HPC] = br.results[c]["attnd"].astype(np.float32)

    out = _host_tail(Q, V, Wv, bv, Wo, bo, attn)
    if _return_results:
        return (out, attn), br
    return out, attn


# revision 59
# speedup vs baseline: 1.0560x; 1.0560x over previous
"""Trainium2 Bass kernel for MultiHeadAttention with physics bias.

Math (per reference):
    q = (Q @ Wq + bq) * scale          # scale folded into Wq/bq host-side
    k = (K @ Wk + bk)
    scores = q @ k^T + lambda * physics_bias
    attn = softmax(scores, axis=-1)    # exp without max-subtraction (scores are O(+-6))
    ctx = attn @ v ; out = Q + ctx @ Wo + bo

Sharding: the 24 (batch, head) pairs are split 3-per-core across 8 cores
(cores 0-3 -> batch 0, cores 4-7 -> batch 1; each takes 3 consecutive heads).
The device computes the O(S^2) part (scores, exp, rowsum, normalize) which is
memory-bound on the [B,H,S,S] bias read + attn write (~100 MB per core).
The O(S) tail (ctx = attn @ v, output projection, residual) is a ~1 GFLOP
epilogue computed on the host from the returned attn.

Device per (head, 128-query chunk):
    PE   : 4x bf16 matmul  q_chunk^T.T @ k^T -> PSUM [128,2048]   (K=5)
    DVE  : scores = (bias * lambda) + psum   (one scalar_tensor_tensor op)
    ACT  : E = exp(scores) in-place, accum_out = row sums
    DVE  : reciprocal; attn = E * (1/sum) in-place
    DMA  : bias in / attn out in 2 MB transfers (two chunks per transfer)
"""

import sys

import numpy as np

try:
    import concourse  # noqa: F401  (already importable via the site setup)
except ImportError:
    _TRN_REPO = "/opt/trn_rl_repo"
    if _TRN_REPO not in sys.path:
        sys.path.insert(0, _TRN_REPO)

B, S, D_MODEL, N_HEADS, D_K = 2, 2048, 64, 12, 5
D_INNER = N_HEADS * D_K          # 60
HPC = 3                          # heads per core
NCORES = 8
DA = D_MODEL + 1                 # augmented with a ones-row to fold biases bq/bk
BANK = 512                       # fp32 PSUM bank / max moving free dim
CHUNK = 128                      # query rows per softmax chunk (= partitions)
GROUP = 2                        # chunks per DMA transfer (2 MB)
GROUP_ROWS = GROUP * CHUNK
NGROUP = S // GROUP_ROWS         # transfers per head

_PROG = None                     # compiled program cache (module-level)


def _build_program():
    import concourse.bacc as bacc
    import concourse.bass as bass
    import concourse.mybir as mybir
    import concourse.tile as tile

    f32 = mybir.dt.float32
    f16 = mybir.dt.float16
    bf16 = mybir.dt.bfloat16
    AF = mybir.ActivationFunctionType
    OP = mybir.AluOpType

    nc = bacc.Bacc("TRN2", target_bir_lowering=False, debug=False,
                   num_devices=NCORES)

    xq = nc.dram_tensor("xq", [DA, S], bf16, kind="ExternalInput").ap()
    xk = nc.dram_tensor("xk", [DA, S], bf16, kind="ExternalInput").ap()
    # per head, the 5 weight columns are pre-replicated at offsets 0/32/64/96
    # so one projection matmul yields q^T/k^T in all four PE row-group slots
    wq = nc.dram_tensor("wq", [DA, HPC * CHUNK], bf16,
                        kind="ExternalInput").ap()
    wk = nc.dram_tensor("wk", [DA, HPC * CHUNK], bf16,
                        kind="ExternalInput").ap()
    # lamdiag[:, 0:128] = lambda * I[128]: bias-add via TensorE diagonal
    # matmul into the scores PSUM; lamdiag[:, 128] = a lambda column for the
    # DVE half of the bias-add (lambda=0.5/1.0 are exact in fp16)
    lamdiag = nc.dram_tensor("lamdiag", [CHUNK, CHUNK + 1], f16,
                             kind="ExternalInput").ap()
    # bias is loaded as fp16: halves the dominant read traffic; fp16's 2^-11
    # mantissa keeps the attn error ~1e-3 (bf16 would be ~1e-2)
    biasd = nc.dram_tensor("biasd", [HPC, S, S], f16, kind="ExternalInput").ap()
    # attn is stored as fp16 (host upcasts): halves the write traffic for
    # another ~5e-4 of rounding on values that are already ~1e-3 approximate
    attnd = nc.dram_tensor("attnd", [HPC, S, S], f16, kind="ExternalOutput").ap()

    with tile.TileContext(nc) as tc:
        with (
            tc.tile_pool(name="const", bufs=1) as const,
            tc.tile_pool(name="spsum", bufs=2, space=bass.MemorySpace.PSUM) as spool,
            tc.tile_pool(name="bias", bufs=8) as bpool,
            tc.tile_pool(name="attn", bufs=8) as apool,
            tc.tile_pool(name="stat", bufs=8) as stat,
        ):
            xq_sb = const.tile([DA, S], bf16, tag="xq", name="xq_sb")
            nc.sync.dma_start(xq_sb[:], xq[:])
            xk_sb = const.tile([DA, S], bf16, tag="xk", name="xk_sb")
            nc.sync.dma_start(xk_sb[:], xk[:])
            wq_sb = const.tile([DA, HPC * CHUNK], bf16, tag="wq", name="wq_sb")
            nc.sync.dma_start(wq_sb[:], wq[:])
            wk_sb = const.tile([DA, HPC * CHUNK], bf16, tag="wk", name="wk_sb")
            nc.sync.dma_start(wk_sb[:], wk[:])
            lam_sb = const.tile([CHUNK, CHUNK + 1], f16, tag="lam",
                                name="lam_sb")
            nc.sync.dma_start(lam_sb[:], lamdiag[:])

            # All q^T/k^T projections upfront (bf16, ~0.4us per matmul) so
            # head transitions never stall the scores/PSUM pipeline. Each is
            # then replicated at partition offsets 0/32/64/96 so the 4 score
            # matmuls of a chunk run in distinct PE row-groups concurrently
            # (their LDWEIGHTS overlap in-flight matmuls of other groups).
            qts, kts = [], []
            for h in range(HPC):
                qt = const.tile([CHUNK, S], bf16, tag=f"qt{h}", name=f"qt{h}")
                kt = const.tile([CHUNK, S], bf16, tag=f"kt{h}", name=f"kt{h}")
                for w_sb, x_sb, dst in ((wq_sb, xq_sb, qt), (wk_sb, xk_sb, kt)):
                    ps = spool.tile([CHUNK, S], f32, tag="ps", name="ps_proj")
                    for j in range(S // BANK):
                        nc.tensor.matmul(
                            ps[:, j * BANK:(j + 1) * BANK],
                            w_sb[:, h * CHUNK:(h + 1) * CHUNK],
                            x_sb[:, j * BANK:(j + 1) * BANK],
                            start=True, stop=True,
                        )
                    nc.vector.tensor_copy(dst[:], ps[:])
                qts.append(qt)
                kts.append(kt)

            for h in range(HPC):
                qt, kt = qts[h], kts[h]
                for pr in range(NGROUP):
                    r0 = pr * GROUP_ROWS
                    bt = bpool.tile([CHUNK, GROUP * S], f16, tag="bt", name="bt")
                    nc.sync.dma_start(
                        bt[:],
                        biasd[h, r0:r0 + GROUP_ROWS, :].rearrange(
                            "(a p) n -> p a n", p=CHUNK),
                    )
                    for a in range(GROUP):
                        ps = spool.tile([CHUNK, S], f32, tag="ps", name="ps_sc")
                        # scores = q_chunk^T.T @ k^T
                        NB = S // BANK
                        PE_B = 2  # banks getting the bias via PE; rest on DVE
                        c0 = (pr * GROUP + a) * CHUNK
                        for j in range(NB):
                            nc.tensor.matmul(
                                ps[:, j * BANK:(j + 1) * BANK],
                                qt[32 * j:32 * j + D_K, c0:c0 + CHUNK],
                                kt[32 * j:32 * j + D_K,
                                   j * BANK:(j + 1) * BANK],
                                start=True, stop=(j >= PE_B),
                                tile_position=(32 * j, 0),
                            )
                        # += lambda * bias: banks 0..PE_B-1 via the diagonal
                        # matmul, the rest via DVE in-place into PSUM — splits
                        # the per-matmul LDWEIGHTS toll between two engines
                        for j in range(PE_B):
                            nc.tensor.matmul(
                                ps[:, j * BANK:(j + 1) * BANK],
                                lam_sb[:, 0:CHUNK],
                                bt[:, a * S + j * BANK:a * S + (j + 1) * BANK],
                                start=False, stop=True,
                            )
                        nc.vector.scalar_tensor_tensor(
                            ps[:, PE_B * BANK:],
                            bt[:, a * S + PE_B * BANK:(a + 1) * S],
                            lam_sb[:, CHUNK:CHUNK + 1],
                            ps[:, PE_B * BANK:],
                            op0=OP.mult, op1=OP.add,
                        )
                        # per-chunk attn tile: Tile tracks deps per tile, so a
                        # shared group tile would serialize chunk a=1 behind
                        # chunk a=0's normalize
                        at = apool.tile([CHUNK, S], f16, tag="at", name="at")
                        sm = stat.tile([CHUNK, 1], f32, tag="sm", name="sm")
                        nc.scalar.activation(at[:], ps[:], AF.Exp,
                                             accum_out=sm[:])
                        rc = stat.tile([CHUNK, 1], f32, tag="rc", name="rc")
                        nc.vector.reciprocal(rc[:], sm[:])
                        nc.vector.tensor_scalar_mul(at[:], at[:], rc[:])
                        # store via GpSimd SWDGE: its own DMA queue (loads
                        # ride the SP queue; fp16 both ways keeps the queues
                        # balanced) and an idle engine's stream (a store's
                        # sem-wait on ACT/SP would block exps / loads behind
                        # it)
                        nc.gpsimd.dma_start(
                            attnd[h, r0 + a * CHUNK:r0 + (a + 1) * CHUNK, :],
                            at[:],
                        )

    nc.compile()
    return nc


def _get_program():
    global _PROG
    if _PROG is None:
        _PROG = _build_program()
    return _PROG


def _make_in_maps(Q, K, physics_bias, lambda_scalar, Wq, bq, Wk, bk):
    import ml_dtypes
    bf16 = ml_dtypes.bfloat16
    scale = float(1.0 / np.sqrt(np.float32(D_K)).astype(np.float32))
    ones_row = np.ones((1, S), np.float32)
    lam_v = np.float32(np.asarray(lambda_scalar))
    lam_diag = np.concatenate(
        [np.eye(CHUNK, dtype=np.float32) * lam_v,
         np.full((CHUNK, 1), lam_v, np.float32)], axis=1).astype(np.float16)

    in_maps = []
    for c in range(NCORES):
        b = c // (N_HEADS // HPC)
        h0 = (c % (N_HEADS // HPC)) * HPC
        cols = slice(h0 * D_K, (h0 + HPC) * D_K)
        wq_aug = np.ascontiguousarray(
            np.vstack([Wq[:, cols], bq[cols][None, :]]) * np.float32(scale),
            np.float32)
        wk_aug = np.ascontiguousarray(
            np.vstack([Wk[:, cols], bk[cols][None, :]]), np.float32)
        # replicate each head's 5 columns at offsets 0/32/64/96 (PE row-groups)
        wq_rep = np.zeros((DA, HPC * CHUNK), np.float32)
        wk_rep = np.zeros((DA, HPC * CHUNK), np.float32)
        for hl in range(HPC):
            for g in range(4):
                dst = slice(hl * CHUNK + 32 * g, hl * CHUNK + 32 * g + D_K)
                src = slice(hl * D_K, (hl + 1) * D_K)
                wq_rep[:, dst] = wq_aug[:, src]
                wk_rep[:, dst] = wk_aug[:, src]
        in_maps.append({
            "xq": np.ascontiguousarray(
                np.vstack([Q[b].T, ones_row]).astype(bf16)),
            "xk": np.ascontiguousarray(
                np.vstack([K[b].T, ones_row]).astype(bf16)),
            "wq": wq_rep.astype(bf16),
            "wk": wk_rep.astype(bf16),
            "lamdiag": lam_diag,
            "biasd": np.ascontiguousarray(
                physics_bias[b, h0:h0 + HPC].astype(np.float16)),
        })
    return in_maps


def _run_device(in_maps, trace=False):
    from concourse.bass_utils import run_bass_kernel_spmd
    nc = _get_program()
    return run_bass_kernel_spmd(nc, in_maps, list(range(NCORES)), trace=trace)


def _host_tail(Q, V, Wv, bv, Wo, bo, attn):
    """ctx = attn @ v per head; out = Q + ctx @ Wo + bo  (all fp32 on host)."""
    v = V.astype(np.float32) @ Wv.astype(np.float32) + bv.astype(np.float32)
    # [B,S,60] -> [B,H,S,5]
    v_heads = np.ascontiguousarray(
        v.reshape(B, S, N_HEADS, D_K).transpose(0, 2, 1, 3))
    ctx = np.matmul(attn, v_heads)                       # [B,H,S,5]
    ctx = ctx.transpose(0, 2, 1, 3).reshape(B, S, D_INNER)
    out = Q.astype(np.float32) + (ctx @ Wo.astype(np.float32)
                                  + bo.astype(np.float32))
    return out.astype(np.float32)


def kernel(Q, K, V, physics_bias, lambda_scalar, Wq, bq, Wk, bk, Wv, bv, Wo, bo,
           _trace=False, _return_results=False):
    Q = np.asarray(Q, np.float32)
    K = np.asarray(K, np.float32)
    V = np.asarray(V, np.float32)
    physics_bias = np.asarray(physics_bias, np.float32)
    Wq = np.asarray(Wq, np.float32)
    bq = np.asarray(bq, np.float32)
    Wk = np.asarray(Wk, np.float32)
    bk = np.asarray(bk, np.float32)
    Wv = np.asarray(Wv, np.float32)
    bv = np.asarray(bv, np.float32)
    Wo = np.asarray(Wo, np.float32)
    bo = np.asarray(bo, np.float32)

    in_maps = _make_in_maps(Q, K, physics_bias, lambda_scalar, Wq, bq, Wk, bk)
    br = _run_device(in_maps, trace=_trace)

    attn = np.empty((B, N_HEADS, S, S), np.float32)
    for c in range(NCORES):
        b = c // (N_HEADS // HPC)
        h0 = (c % (N_HEADS // HPC)) * HPC
        attn[b, h0:h0 + HPC] = br.results[c]["attnd"].astype(np.float32)

    out = _host_tail(Q, V, Wv, bv, Wo, bo, attn)
    if _return_results:
        return (out, attn), br
    return out, attn
